# revision 24
# baseline (speedup 1.0000x reference)
"""Trainium2 Bass kernel for a causal self-attention transformer block.

Reference computation (per batch b):
    qkv = x @ w_qkv.T ; split into q, k, v heads (16 heads, dim 64)
    s   = (q @ k.T) * dh**-0.5, causal + padding mask
    a   = softmax(s, axis=j)
    o   = (a @ v) @ w_out.T + b_out ; out = o * m[:, None]

Sharding: pure data parallel — batch (8) across the 8 NeuronCores, weights
replicated. No collectives.

Per-core device program (v3):
  - host-pre-transposed inputs: xT [d, t], wqk tiled [16, 8, 128, 128]
    (lhsT tiles), wv/wo [d, e]; all matmul operands fp16, accumulation fp32
    in PSUM.
  - qT/kT in [e, t] layout (2 heads per 128-partition tile); v in [t, e]
    layout with a padding-mask column so A@V also emits the softmax
    denominator row.
  - scores transposed: S_T[j, i] = K^T.T @ Q^T; the two heads' K=64 matmuls
    stream concurrently in row-groups 0-1/2-3 into one 2-bank PSUM tile
    (double-buffered), exp is a single ACTIVATE per chunk over both heads,
    causal tri-mask one DVE multiply per j-tile. The score loop is filler-
    woven with the next pair's q/k projection because exp paces it.
  - A@V dense blocks, hh-outer J-major: 2 live accumulators, stationary
    shared across the two i-chunks per (head, J) so half the LDWEIGHTS
    disappear; denominator rows staged on partition 0, tiny DMA to
    partitions 0/1, reciprocal_approx_fast, fp16 cast; normalization
    broadcast via a K=2 fp16 sel2-matmul one pair late.
  - pair 7 has no next projection to weave, so it weaves the first out-proj
    t-tile AND partial out-proj accumulations (pairs 0-5) for t-tiles 1-3,
    evacuated masked to SBUF; phase 3 resumes them with a fused
    (psum*mask)+partial DVE op. Pair 6/7 oT normalization is chunked so
    out-proj starts immediately — the PE never idles long enough for the
    HAM clock-gate to re-throttle.
  - b_out is added on the host (removes 16 fp32r K=1 bias matmuls).
"""

import os
import numpy as np
from contextlib import ExitStack

import ml_dtypes
from concourse import bacc
import concourse.mybir as mybir
import concourse.tile as tile
from concourse.bass_utils import run_bass_kernel_spmd

D = 1024          # model dim
T = 1024          # sequence length
H = 16            # heads
DH = 64           # head dim
P = 128           # partitions
ND = D // P       # d-tiles
NT = T // P       # t-tiles
NPAIR = H // 2    # head pairs
SCALE = DH ** -0.5
F32 = mybir.dt.float32
MULT = mybir.AluOpType.mult
ADD = mybir.AluOpType.add
EXP = mybir.ActivationFunctionType.Exp

_MM_MODE = os.environ.get("TRN_MM_DT", "fp16")
MM_DT = {"fp16": mybir.dt.float16, "bf16": mybir.dt.bfloat16}[_MM_MODE]
NP_MM = {"fp16": np.float16, "bf16": ml_dtypes.bfloat16}[_MM_MODE]

_CACHE = {}
LAST_RESULTS = None
PART_TTS = (1, 2, 3)   # t-tiles whose pair-0..5 out-proj is woven into pair 7


def _qk_chunks(J):
    """i-column chunks (lo, width) of computed scores for j-tile J."""
    out = []
    for lo in (J * P, J * P + 512):
        w = min(512, T - lo)
        if w > 0:
            out.append((lo, w))
    return out


def _av_ranges(J):
    """(ci, lo, width) A@V output ranges for j-tile J."""
    out = []
    if J < 4:
        out.append((0, J * P, 512 - J * P))
    lo = max(512, J * P)
    out.append((1, lo, T - lo))
    return out


def _emit(nc, tc, xT_d, wqk_d, wv_d, wo_d, mcol_d, tri_d, sel2_d, out_d):
    ctx = ExitStack()
    with ctx:
        const = ctx.enter_context(tc.tile_pool(name="const", bufs=1))
        xt_p = ctx.enter_context(tc.tile_pool(name="xt", bufs=1))
        vaug_p = ctx.enter_context(tc.tile_pool(name="vaug", bufs=1))
        qkT_p = ctx.enter_context(tc.tile_pool(name="qkT", bufs=2))
        wqk_p = ctx.enter_context(tc.tile_pool(name="wqk", bufs=4))
        pt_p = ctx.enter_context(tc.tile_pool(name="pt", bufs=10))
        oT_p = ctx.enter_context(tc.tile_pool(name="oT", bufs=1))
        wv_p = ctx.enter_context(tc.tile_pool(name="wv", bufs=1))
        wo_p = ctx.enter_context(tc.tile_pool(name="wo", bufs=1))
        osb_p = ctx.enter_context(tc.tile_pool(name="osb", bufs=6))
        part_p = ctx.enter_context(tc.tile_pool(name="part", bufs=6))
        den_p = ctx.enter_context(tc.tile_pool(name="den", bufs=2))
        # PSUM: 8 banks = psA(2) + psS(2x2) + psAV(2)
        psA = ctx.enter_context(tc.tile_pool(name="psA", bufs=2, space="PSUM"))
        psS = ctx.enter_context(tc.tile_pool(name="psS", bufs=2, space="PSUM"))
        psAV = ctx.enter_context(tc.tile_pool(name="psAV", bufs=2, space="PSUM"))

        # resident xT and wv tiles, one tile per d-tile (separate tiles give
        # per-DMA dependency granularity so the first V-proj matmul only
        # waits for its own d-slice), DMA'd interleaved in consumption order.
        xT_r = xT_d.ap().rearrange("(n p) t -> p n t", p=P)
        wv_r = wv_d.ap().rearrange("(n p) t -> p n t", p=P)
        xt_tiles = [xt_p.tile([P, T], MM_DT, tag=f"xt{d}", name=f"xt{d}")
                    for d in range(ND)]
        wv_tiles = [wv_p.tile([P, T], MM_DT, tag=f"wv{d}", name=f"wv{d}")
                    for d in range(ND)]
        # xt on the Sync DGE queue, wv on the Scalar DGE queue: the two
        # hardware DMA queues run concurrently so the V-projection's d-tile
        # pairs arrive ~2x faster at startup
        for q in range(ND):
            nc.sync.dma_start(out=xt_tiles[q][:], in_=xT_r[:, q, :])
            nc.scalar.dma_start(out=wv_tiles[q][:], in_=wv_r[:, q, :])
        xts = [xt_tiles[d][:] for d in range(ND)]
        wvts = [wv_tiles[d][:] for d in range(ND)]

        mcol = const.tile([P, NT], F32, tag="mcol", name="mcol")
        nc.sync.dma_start(out=mcol[:], in_=mcol_d.ap())
        tri = const.tile([P, P], MM_DT, tag="tri", name="tri")
        nc.sync.dma_start(out=tri[:], in_=tri_d.ap())
        sel2 = const.tile([2, P], MM_DT, tag="sel2", name="sel2")
        nc.sync.dma_start(out=sel2[:], in_=sel2_d.ap())

        vaug = [
            vaug_p.tile([P, H, DH + 1], MM_DT, tag=f"va{t}", name=f"va{t}")
            for t in range(NT)
        ]

        # ---- Phase 1: V projection (natural layout).
        for g2 in range(0, NT, 2):
            accs = {}
            for i in range(2):
                for c in range(2):
                    pool = psA if i == 0 else psAV
                    accs[i, c] = pool.tile(
                        [P, 512], F32, tag=("ps" if i == 0 else "av"),
                        name=f"vps{i}{c}",
                    )
            for d in range(ND):
                for i in range(2):
                    tt = g2 + i
                    for c in range(2):
                        nc.tensor.matmul(
                            accs[i, c][:],
                            xts[d][:, tt * P:(tt + 1) * P],
                            wvts[d][:, c * 512:(c + 1) * 512],
                            start=(d == 0),
                            stop=(d == ND - 1),
                        )
            for i in range(2):
                tt = g2 + i
                for c in range(2):
                    ps3 = accs[i, c][:].rearrange("p (h e) -> p h e", e=DH)
                    nc.vector.tensor_scalar(
                        vaug[tt][:, c * 8:(c + 1) * 8, 0:DH],
                        ps3,
                        mcol[:, tt:tt + 1],
                        None,
                        MULT,
                    )
        for tt in range(NT):
            nc.vector.tensor_copy(
                out=vaug[tt][:, :, DH],
                in_=mcol[:, tt:tt + 1].to_broadcast([P, H]),
            )

        # ---- Phase 2: per head-pair: q/k projection then attention.
        def _proj(g, qT, kT, wide=False):
            """Generator emitting pair g's q/k projection in small steps.
            wide=True interleaves the q and k chains on 4 accumulators
            (only legal when psAV is otherwise idle)."""
            if wide:
                wts, pss = [], []
                for et in (g, NPAIR + g):
                    wt = wqk_p.tile([P, ND, P], MM_DT, tag="wqk", name="wqkt")
                    nc.sync.dma_start(
                        out=wt[:], in_=wqk_d.ap()[et].rearrange("n p e -> p n e")
                    )
                    wts.append(wt)
                    pool = psA if et == g else psAV
                    tg = "ps" if et == g else "av"
                    pss.append([
                        pool.tile([P, 512], F32, tag=tg, name="qkps")
                        for _ in range(2)
                    ])
                for d in range(ND):
                    for k in range(2):
                        for half in range(2):
                            nc.tensor.matmul(
                                pss[k][half][:],
                                wts[k][:, d, :],
                                xts[d][:, half * 512:(half + 1) * 512],
                                start=(d == 0), stop=(d == ND - 1),
                            )
                    yield "d"
                for k, dest in ((0, qT), (1, kT)):
                    for half in range(2):
                        nc.vector.tensor_copy(
                            out=dest[:, half * 512:(half + 1) * 512],
                            in_=pss[k][half][:],
                        )
                    yield "dest"
                return
            for dest, et in ((qT, g), (kT, NPAIR + g)):
                wt = wqk_p.tile([P, ND, P], MM_DT, tag="wqk", name="wqkt")
                nc.sync.dma_start(
                    out=wt[:], in_=wqk_d.ap()[et].rearrange("n p e -> p n e")
                )
                ps0 = psA.tile([P, 512], F32, tag="ps", name="qkps0")
                ps1 = psA.tile([P, 512], F32, tag="ps", name="qkps1")
                for d in range(ND):
                    nc.tensor.matmul(
                        ps0[:], wt[:, d, :], xts[d][:, 0:512],
                        start=(d == 0), stop=(d == ND - 1),
                    )
                    nc.tensor.matmul(
                        ps1[:], wt[:, d, :], xts[d][:, 512:1024],
                        start=(d == 0), stop=(d == ND - 1),
                    )
                    yield "d"
                nc.vector.tensor_copy(out=dest[:, 0:512], in_=ps0[:])
                nc.vector.tensor_copy(out=dest[:, 512:1024], in_=ps1[:])
                yield "dest"

        def _pull(it, n):
            for _ in range(n):
                try:
                    next(it)
                except StopIteration:
                    return

        oTs = []
        qkTs = {0: (
            qkT_p.tile([P, T], MM_DT, tag="qT", name="qT0"),
            qkT_p.tile([P, T], MM_DT, tag="kT", name="kT0"),
        )}
        _pull(_proj(0, *qkTs[0], wide=True), 99)

        wo_all = wo_p.tile([P, NPAIR, T], MM_DT, tag="wo", name="wot")
        wo_r = wo_d.ap().rearrange("(n p) t -> p n t", p=P)
        for q in range(4):
            nc.scalar.dma_start(
                out=wo_all[:, 2 * q:2 * q + 2, :], in_=wo_r[:, 2 * q:2 * q + 2, :]
            )
        wots = [wo_all[:, g, :] for g in range(NPAIR)]
        op_accs = None
        rcps = {}
        parts = {}

        def _norm_chunk(oT, rcpg, c, bc):
            nc.tensor.matmul(
                bc[:], sel2[:], rcpg[c][0:2, 0:512],
                start=True, stop=True,
            )
            nc.vector.tensor_tensor(
                oT[:, c * 512:(c + 1) * 512],
                oT[:, c * 512:(c + 1) * 512],
                bc[:],
                MULT,
            )

        def _p7_weave():
            # out-proj t-tile 0, pairs 0..5
            for gg in range(6):
                for c in range(2):
                    nc.tensor.matmul(
                        op_accs[c][:],
                        oTs[gg][:, 0:P],
                        wots[gg][:, c * 512:(c + 1) * 512],
                        start=(gg == 0), stop=False,
                    )
                yield "op"

            def _partial(ptt, c):
                pb = psAV.tile([P, 512], F32, tag="av", name=f"pb{ptt}{c}")
                for gg in range(6):
                    nc.tensor.matmul(
                        pb[:],
                        oTs[gg][:, ptt * P:(ptt + 1) * P],
                        wots[gg][:, c * 512:(c + 1) * 512],
                        start=(gg == 0), stop=(gg == 5),
                    )
                sb = part_p.tile([P, 512], F32, tag="part", name=f"pt{ptt}{c}")
                parts[ptt, c] = sb
                nc.vector.tensor_scalar(
                    sb[:], pb[:], mcol[:, ptt:ptt + 1], None, MULT,
                )

            # partial out-proj (pairs 0..5, masked) for t-tiles 1..3, with
            # pair-6 normalization + its t-tile-0 matmuls slotted between
            _partial(PART_TTS[0], 0)
            yield "p"
            _partial(PART_TTS[0], 1)
            yield "p"
            for c in range(2):
                bc = psAV.tile([P, 512], F32, tag="av", name=f"nbc6_{c}")
                _norm_chunk(oTs[6], rcps[6], c, bc)
            yield "n6"
            _partial(PART_TTS[1], 0)
            yield "p"
            _partial(PART_TTS[1], 1)
            yield "p"
            for c in range(2):
                nc.tensor.matmul(
                    op_accs[c][:],
                    oTs[6][:, 0:P],
                    wots[6][:, c * 512:(c + 1) * 512],
                    start=False, stop=False,
                )
            yield "op6"
            _partial(PART_TTS[2], 0)
            yield "p"
            _partial(PART_TTS[2], 1)
            yield "p"

        for g in range(NPAIR):
            qT, kT = qkTs[g]
            last = g == NPAIR - 1
            if not last:
                qkTs[g + 1] = (
                    qkT_p.tile([P, T], MM_DT, tag="qT", name=f"qT{g + 1}"),
                    qkT_p.tile([P, T], MM_DT, tag="kT", name=f"kT{g + 1}"),
                )
                nxt = _proj(g + 1, *qkTs[g + 1])
            else:
                op_accs = {
                    c: psA.tile([P, 512], F32, tag="ps", name=f"ops0_{c}")
                    for c in range(2)
                }
                nxt = _p7_weave()

            oT = oT_p.tile([P, T], MM_DT, tag=f"oT{g}", name=f"oT{g}")
            oTs.append(oT)
            # separate tiles per ci-chunk: chunk 0's normalize matmul must
            # not pick up a dependency on chunk 1's reciprocal chain
            deng = {ci: den_p.tile([1, 2, 512], F32, tag=f"deng{ci}",
                                   name=f"deng{g}_{ci}") for ci in (0, 1)}
            den2 = {ci: den_p.tile([2, 512], F32, tag=f"den2_{ci}",
                                   name=f"den2_{g}_{ci}") for ci in (0, 1)}
            rf32 = {ci: den_p.tile([2, 512], F32, tag=f"rf32_{ci}",
                                   name=f"rf32_{g}_{ci}") for ci in (0, 1)}
            rcpg = {ci: den_p.tile([2, 512], MM_DT, tag=f"rcp{ci}",
                                   name=f"rcp{g}_{ci}") for ci in (0, 1)}
            rcps[g] = rcpg

            # dense score block, exp-paced: weave fills the ACT gaps
            ptts = {}
            for J in range(NT):
                ptts[J] = pt_p.tile([P, 2, T], MM_DT, tag="pt", name=f"pt{J}")
                for (lo, w) in _qk_chunks(J):
                    ps = psS.tile([P, 1024], F32, tag="s", name="sps")
                    for hh in (0, 1):
                        hs = slice(hh * DH, (hh + 1) * DH)
                        nc.tensor.matmul(
                            ps[:, hh * 512:hh * 512 + w],
                            kT[hs, J * P:(J + 1) * P],
                            qT[hs, lo:lo + w],
                            start=True, stop=True,
                        )
                    nc.scalar.activation(
                        out=ptts[J][:, :, lo:lo + w],
                        in_=ps[:].rearrange("p (h i) -> p h i", h=2)[:, :, :w],
                        func=EXP, scale=SCALE,
                    )
                    _pull(nxt, 1)
                nc.vector.tensor_tensor(
                    ptts[J][:, :, J * P:(J + 1) * P],
                    ptts[J][:, :, J * P:(J + 1) * P],
                    tri[:].rearrange("p (o j) -> p o j", o=1)
                          .to_broadcast([P, 2, P]),
                    MULT,
                )
                if J < 6:
                    _pull(nxt, 1)

            # dense A@V, ci-outer like the baseline (2 rotating banks, the
            # bank-reuse WAR is one whole block behind its evacuation). oT
            # evacuation on ACT (it idles once the exps drain), denominator
            # rows + reciprocal per ci-chunk on DVE.
            def _recip_chunk(ci):
                nc.sync.dma_start(out=den2[ci][:], in_=deng[ci][0:1, :, :])
                nc.vector.reciprocal_approx_fast(
                    out=rf32[ci][:], in_=den2[ci][:]
                )
                with nc.allow_low_precision(reason="fp16 recip for matmul"):
                    nc.vector.tensor_copy(out=rcpg[ci][:], in_=rf32[ci][:])

            def _av_block(hh, ci):
                h = 2 * g + hh
                clo = ci * 512
                acc = psAV.tile([P, 512], F32, tag="av", name=f"av{hh}{ci}")
                Js = range(4) if ci == 0 else range(NT)
                for J in Js:
                    lo = max(clo, J * P)
                    w = clo + 512 - lo
                    nc.tensor.matmul(
                        acc[0:DH + 1, lo - clo:lo - clo + w],
                        vaug[J][:, h, :],
                        ptts[J][:, hh, lo:lo + w],
                        start=(J == 0), stop=(J == Js[-1]),
                    )
                return acc

            def _av_evac(acc, hh, ci, on_act):
                # the bank-reuse WAR gates later PE work through this
                # evacuation, so it goes on whichever engine is idle at this
                # point of the pair: DVE mid-pair (ACT still drains exps),
                # ACT at pair end (exps done, DVE has the recip chain)
                hs = slice(hh * DH, (hh + 1) * DH)
                clo = ci * 512
                if on_act:
                    nc.scalar.copy(
                        out=oT[hs, clo:clo + 512], in_=acc[0:DH, 0:512],
                    )
                else:
                    nc.vector.tensor_copy(
                        out=oT[hs, clo:clo + 512], in_=acc[0:DH, 0:512],
                    )
                nc.vector.tensor_copy(
                    out=deng[ci][0:1, hh, 0:512],
                    in_=acc[DH:DH + 1, 0:512],
                )

            acc = _av_block(0, 0)
            _av_evac(acc, 0, 0, on_act=False)
            _pull(nxt, 1)
            acc = _av_block(1, 0)
            _av_evac(acc, 1, 0, on_act=False)
            _recip_chunk(0)
            _pull(nxt, 1)
            acc = _av_block(0, 1)
            if last:
                # normalize pair 7's first oT chunk NOW: its inputs (ci=0
                # evacuations + reciprocal) are long done, and emitting it
                # before av01's evacuation keeps its counting-sem thresholds
                # low so the in-order PE queue never stalls here
                bc = psAV.tile([P, 512], F32, tag="av", name="nbc7_0")
                _norm_chunk(oT, rcps[7], 0, bc)
            _av_evac(acc, 0, 1, on_act=True)
            _pull(nxt, 1)
            acc = _av_block(1, 1)
            _av_evac(acc, 1, 1, on_act=True)
            _recip_chunk(1)
            _pull(nxt, 99)

            # normalize the PREVIOUS pair (reciprocal long ready; pair 7
            # normalizes pair 6 inline in its weave instead)
            if 1 <= g < NPAIR - 1:
                for c in range(2):
                    bc = psA.tile([P, 512], F32, tag="ps", name=f"nbc{g}_{c}")
                    _norm_chunk(oTs[g - 1], rcps[g - 1], c, bc)

        # ---- Phase 3: output projection.
        def _op_finish(tt, accs):
            for c in range(2):
                osb = osb_p.tile([P, 512], F32, tag="osb", name="osb")
                if (tt, c) in parts:
                    nc.vector.scalar_tensor_tensor(
                        out=osb[:], in0=accs[c][:], scalar=mcol[:, tt:tt + 1],
                        in1=parts[tt, c][:], op0=MULT, op1=ADD,
                    )
                else:
                    nc.vector.tensor_scalar(
                        osb[:], accs[c][:], mcol[:, tt:tt + 1], None, MULT,
                    )
                nc.sync.dma_start(
                    out=out_d.ap()[tt * P:(tt + 1) * P,
                                   c * 512:(c + 1) * 512],
                    in_=osb[:],
                )

        def _op_alloc():
            return {
                c: (psA if c == 0 else psAV).tile(
                    [P, 512], F32, tag=("ps" if c == 0 else "av"),
                    name=f"ops{c}",
                )
                for c in range(2)
            }

        # t-tile 0 finishes immediately (oT7 cols < 128 normalized in-pair)
        for c in range(2):
            nc.tensor.matmul(
                op_accs[c][:], oTs[7][:, 0:P],
                wots[7][:, c * 512:(c + 1) * 512],
                start=False, stop=True,
            )
        _op_finish(0, op_accs)

        # partial t-tiles: resume with pairs 6..7 only
        for tt in PART_TTS:
            accs = _op_alloc()
            for gg in (6, 7):
                for c in range(2):
                    nc.tensor.matmul(
                        accs[c][:],
                        oTs[gg][:, tt * P:(tt + 1) * P],
                        wots[gg][:, c * 512:(c + 1) * 512],
                        start=(gg == 6), stop=(gg == 7),
                    )
            _op_finish(tt, accs)

        bc = psAV.tile([P, 512], F32, tag="av", name="nbc7_1")
        _norm_chunk(oTs[7], rcps[7], 1, bc)

        for tt in range(PART_TTS[-1] + 1, NT):
            accs = _op_alloc()
            for gg in range(NPAIR):
                for c in range(2):
                    nc.tensor.matmul(
                        accs[c][:],
                        oTs[gg][:, tt * P:(tt + 1) * P],
                        wots[gg][:, c * 512:(c + 1) * 512],
                        start=(gg == 0), stop=(gg == NPAIR - 1),
                    )
            _op_finish(tt, accs)


def build_nc():
    nc = bacc.Bacc("TRN2", target_bir_lowering=False, debug=False,
                   num_devices=8)
    xT_d = nc.dram_tensor("xT", [D, T], MM_DT, kind="ExternalInput")
    wqk_d = nc.dram_tensor("wqk", [H, ND, P, P], MM_DT, kind="ExternalInput")
    wv_d = nc.dram_tensor("wv", [D, D], MM_DT, kind="ExternalInput")
    wo_d = nc.dram_tensor("wo", [D, D], MM_DT, kind="ExternalInput")
    mcol_d = nc.dram_tensor("mcol", [P, NT], F32, kind="ExternalInput")
    tri_d = nc.dram_tensor("tri", [P, P], MM_DT, kind="ExternalInput")
    sel2_d = nc.dram_tensor("sel2", [2, P], MM_DT, kind="ExternalInput")
    out_d = nc.dram_tensor("out", [T, D], F32, kind="ExternalOutput")
    with tile.TileContext(nc) as tc:
        _emit(nc, tc, xT_d, wqk_d, wv_d, wo_d, mcol_d, tri_d, sel2_d, out_d)
    nc.compile()
    return nc


def _prep_shared(w_qkv, w_out):
    wqkT = np.ascontiguousarray(w_qkv[:2 * D].T)             # [d, e]
    wqk_tiles = np.ascontiguousarray(
        wqkT.reshape(ND, P, H, P).transpose(2, 0, 1, 3)
    ).astype(NP_MM)                                          # [16, 8, 128, 128]
    wv = np.ascontiguousarray(w_qkv[2 * D:].T).astype(NP_MM)  # [d, ev]
    wo = np.ascontiguousarray(w_out.T).astype(NP_MM)          # [d', e]
    tri = np.triu(np.ones((P, P), dtype=np.float32)).astype(NP_MM)
    sel2 = np.zeros((2, P), dtype=np.float32)
    sel2[0, 0:DH] = 1.0
    sel2[1, DH:P] = 1.0
    sel2 = sel2.astype(NP_MM)
    return wqk_tiles, wv, wo, tri, sel2


def kernel(x, m, w_qkv, w_out, b_out, l=None, **_unused):
    global LAST_RESULTS
    x = np.asarray(x, dtype=np.float32)
    m = np.asarray(m, dtype=np.float32)
    w_qkv = np.asarray(w_qkv, dtype=np.float32)
    w_out = np.asarray(w_out, dtype=np.float32)
    b_out = np.asarray(b_out, dtype=np.float32)

    if "nc" not in _CACHE:
        _CACHE["nc"] = build_nc()
    nc = _CACHE["nc"]

    wqk_tiles, wv, wo, tri, sel2 = _prep_shared(w_qkv, w_out)
    in_maps = []
    for b in range(8):
        in_maps.append({
            "xT": np.ascontiguousarray(x[b].T).astype(NP_MM),
            "wqk": wqk_tiles,
            "wv": wv,
            "wo": wo,
            "mcol": np.ascontiguousarray(m[b].reshape(NT, P).T),
            "tri": tri,
            "sel2": sel2,
        })

    trace = bool(int(os.environ.get("TRN_TRACE", "0")))
    res = run_bass_kernel_spmd(
        nc, in_maps, core_ids=list(range(8)), trace=trace,
    )
    LAST_RESULTS = res
    out = np.stack([res.results[b]["out"] for b in range(8)], axis=0)
    # bias is a broadcast add; do it on the host (masked like the reference)
    out = out + b_out[None, None, :] * m[:, :, None]
    return out.astype(np.float32)


# revision 27
# speedup vs baseline: 1.1943x; 1.1943x over previous
"""Trainium2 Bass kernel for a causal self-attention transformer block.

Reference computation (per batch b):
    qkv = x @ w_qkv.T ; split into q, k, v heads (16 heads, dim 64)
    s   = (q @ k.T) * dh**-0.5, causal + padding mask
    a   = softmax(s, axis=j)
    o   = (a @ v) @ w_out.T + b_out ; out = o * m[:, None]

Sharding: pure data parallel — batch (8) across the 8 NeuronCores, weights
replicated. No collectives.

Per-core device program (v3):
  - host-pre-transposed inputs: xT [d, t], wqk tiled [16, 8, 128, 128]
    (lhsT tiles), wv/wo [d, e]; all matmul operands fp16, accumulation fp32
    in PSUM.
  - qT/kT in [e, t] layout (2 heads per 128-partition tile); v in [t, e]
    layout with a padding-mask column so A@V also emits the softmax
    denominator row.
  - scores transposed: S_T[j, i] = K^T.T @ Q^T; the two heads' K=64 matmuls
    stream concurrently in row-groups 0-1/2-3 into one 2-bank PSUM tile
    (double-buffered), exp is a single ACTIVATE per chunk over both heads,
    causal tri-mask one DVE multiply per j-tile. The score loop is filler-
    woven with the next pair's q/k projection because exp paces it.
  - A@V dense blocks, hh-outer J-major: 2 live accumulators, stationary
    shared across the two i-chunks per (head, J) so half the LDWEIGHTS
    disappear; denominator rows staged on partition 0, tiny DMA to
    partitions 0/1, reciprocal_approx_fast, fp16 cast; normalization
    broadcast via a K=2 fp16 sel2-matmul one pair late.
  - pair 7 has no next projection to weave, so it weaves the first out-proj
    t-tile AND partial out-proj accumulations (pairs 0-5) for t-tiles 1-3,
    evacuated masked to SBUF; phase 3 resumes them with a fused
    (psum*mask)+partial DVE op. Pair 6/7 oT normalization is chunked so
    out-proj starts immediately — the PE never idles long enough for the
    HAM clock-gate to re-throttle.
  - b_out is added on the host (removes 16 fp32r K=1 bias matmuls).
"""

import os
import numpy as np
from contextlib import ExitStack

import ml_dtypes
from concourse import bacc
import concourse.mybir as mybir
import concourse.tile as tile
from concourse.bass_utils import run_bass_kernel_spmd

D = 1024          # model dim
T = 1024          # sequence length
H = 16            # heads
DH = 64           # head dim
P = 128           # partitions
ND = D // P       # d-tiles
NT = T // P       # t-tiles
NPAIR = H // 2    # head pairs
SCALE = DH ** -0.5
F32 = mybir.dt.float32
MULT = mybir.AluOpType.mult
ADD = mybir.AluOpType.add
EXP = mybir.ActivationFunctionType.Exp

_MM_MODE = os.environ.get("TRN_MM_DT", "fp16")
MM_DT = {"fp16": mybir.dt.float16, "bf16": mybir.dt.bfloat16}[_MM_MODE]
NP_MM = {"fp16": np.float16, "bf16": ml_dtypes.bfloat16}[_MM_MODE]

_CACHE = {}
LAST_RESULTS = None
PART_TTS = (1, 2, 3)   # t-tiles whose pair-0..5 out-proj is woven into pair 7


def _qk_chunks(J):
    """i-column chunks (lo, width) of computed scores for j-tile J."""
    out = []
    for lo in (J * P, J * P + 512):
        w = min(512, T - lo)
        if w > 0:
            out.append((lo, w))
    return out


def _av_ranges(J):
    """(ci, lo, width) A@V output ranges for j-tile J."""
    out = []
    if J < 4:
        out.append((0, J * P, 512 - J * P))
    lo = max(512, J * P)
    out.append((1, lo, T - lo))
    return out


def _emit(nc, tc, xT_d, wqk_d, wv_d, wo_d, mcol_d, tri_d, sel2_d, out_d):
    ctx = ExitStack()
    with ctx:
        const = ctx.enter_context(tc.tile_pool(name="const", bufs=1))
        xt_p = ctx.enter_context(tc.tile_pool(name="xt", bufs=1))
        vaug_p = ctx.enter_context(tc.tile_pool(name="vaug", bufs=1))
        qkT_p = ctx.enter_context(tc.tile_pool(name="qkT", bufs=2))
        wqk_p = ctx.enter_context(tc.tile_pool(name="wqk", bufs=4))
        pt_p = ctx.enter_context(tc.tile_pool(name="pt", bufs=10))
        oT_p = ctx.enter_context(tc.tile_pool(name="oT", bufs=1))
        wv_p = ctx.enter_context(tc.tile_pool(name="wv", bufs=1))
        wo_p = ctx.enter_context(tc.tile_pool(name="wo", bufs=1))
        osb_p = ctx.enter_context(tc.tile_pool(name="osb", bufs=6))
        part_p = ctx.enter_context(tc.tile_pool(name="part", bufs=6))
        den_p = ctx.enter_context(tc.tile_pool(name="den", bufs=2))
        # PSUM: 8 banks = psA(2) + psS(2x2) + psAV(2)
        psA = ctx.enter_context(tc.tile_pool(name="psA", bufs=2, space="PSUM"))
        psS = ctx.enter_context(tc.tile_pool(name="psS", bufs=2, space="PSUM"))
        psAV = ctx.enter_context(tc.tile_pool(name="psAV", bufs=2, space="PSUM"))

        # resident xT and wv tiles, one tile per d-tile (separate tiles give
        # per-DMA dependency granularity so the first V-proj matmul only
        # waits for its own d-slice), DMA'd interleaved in consumption order.
        xT_r = xT_d.ap().rearrange("(n p) t -> p n t", p=P)
        wv_r = wv_d.ap().rearrange("(n p) t -> p n t", p=P)
        xt_tiles = [xt_p.tile([P, T], MM_DT, tag=f"xt{d}", name=f"xt{d}")
                    for d in range(ND)]
        wv_tiles = [wv_p.tile([P, T], MM_DT, tag=f"wv{d}", name=f"wv{d}")
                    for d in range(ND)]
        for q in range(ND):
            nc.sync.dma_start(out=xt_tiles[q][:], in_=xT_r[:, q, :])
            nc.sync.dma_start(out=wv_tiles[q][:], in_=wv_r[:, q, :])
        xts = [xt_tiles[d][:] for d in range(ND)]
        wvts = [wv_tiles[d][:] for d in range(ND)]

        mcol = const.tile([P, NT], F32, tag="mcol", name="mcol")
        nc.sync.dma_start(out=mcol[:], in_=mcol_d.ap())
        tri = const.tile([P, P], MM_DT, tag="tri", name="tri")
        nc.sync.dma_start(out=tri[:], in_=tri_d.ap())
        sel2 = const.tile([2, P], MM_DT, tag="sel2", name="sel2")
        nc.sync.dma_start(out=sel2[:], in_=sel2_d.ap())

        vaug = [
            vaug_p.tile([P, H, DH + 1], MM_DT, tag=f"va{t}", name=f"va{t}")
            for t in range(NT)
        ]

        # ---- Phase 1: V projection (natural layout).
        for g2 in range(0, NT, 2):
            accs = {}
            for i in range(2):
                for c in range(2):
                    pool = psA if i == 0 else psAV
                    accs[i, c] = pool.tile(
                        [P, 512], F32, tag=("ps" if i == 0 else "av"),
                        name=f"vps{i}{c}",
                    )
            for d in range(ND):
                for i in range(2):
                    tt = g2 + i
                    for c in range(2):
                        nc.tensor.matmul(
                            accs[i, c][:],
                            xts[d][:, tt * P:(tt + 1) * P],
                            wvts[d][:, c * 512:(c + 1) * 512],
                            start=(d == 0),
                            stop=(d == ND - 1),
                        )
            for i in range(2):
                tt = g2 + i
                for c in range(2):
                    ps3 = accs[i, c][:].rearrange("p (h e) -> p h e", e=DH)
                    nc.vector.tensor_scalar(
                        vaug[tt][:, c * 8:(c + 1) * 8, 0:DH],
                        ps3,
                        mcol[:, tt:tt + 1],
                        None,
                        MULT,
                    )
        for tt in range(NT):
            nc.vector.tensor_copy(
                out=vaug[tt][:, :, DH],
                in_=mcol[:, tt:tt + 1].to_broadcast([P, H]),
            )

        # ---- Phase 2: per head-pair: q/k projection then attention.
        def _proj(g, qT, kT, wide=False):
            """Generator emitting pair g's q/k projection in small steps.
            wide=True interleaves the q and k chains on 4 accumulators
            (only legal when psAV is otherwise idle)."""
            if wide:
                wts, pss = [], []
                for et in (g, NPAIR + g):
                    wt = wqk_p.tile([P, ND, P], MM_DT, tag="wqk", name="wqkt")
                    nc.sync.dma_start(
                        out=wt[:], in_=wqk_d.ap()[et].rearrange("n p e -> p n e")
                    )
                    wts.append(wt)
                    pool = psA if et == g else psAV
                    tg = "ps" if et == g else "av"
                    pss.append([
                        pool.tile([P, 512], F32, tag=tg, name="qkps")
                        for _ in range(2)
                    ])
                for d in range(ND):
                    for k in range(2):
                        for half in range(2):
                            nc.tensor.matmul(
                                pss[k][half][:],
                                wts[k][:, d, :],
                                xts[d][:, half * 512:(half + 1) * 512],
                                start=(d == 0), stop=(d == ND - 1),
                            )
                    yield "d"
                for k, dest in ((0, qT), (1, kT)):
                    for half in range(2):
                        nc.vector.tensor_copy(
                            out=dest[:, half * 512:(half + 1) * 512],
                            in_=pss[k][half][:],
                        )
                    yield "dest"
                return
            for dest, et in ((qT, g), (kT, NPAIR + g)):
                wt = wqk_p.tile([P, ND, P], MM_DT, tag="wqk", name="wqkt")
                nc.sync.dma_start(
                    out=wt[:], in_=wqk_d.ap()[et].rearrange("n p e -> p n e")
                )
                ps0 = psA.tile([P, 512], F32, tag="ps", name="qkps0")
                ps1 = psA.tile([P, 512], F32, tag="ps", name="qkps1")
                for d in range(ND):
                    nc.tensor.matmul(
                        ps0[:], wt[:, d, :], xts[d][:, 0:512],
                        start=(d == 0), stop=(d == ND - 1),
                    )
                    nc.tensor.matmul(
                        ps1[:], wt[:, d, :], xts[d][:, 512:1024],
                        start=(d == 0), stop=(d == ND - 1),
                    )
                    yield "d"
                nc.vector.tensor_copy(out=dest[:, 0:512], in_=ps0[:])
                nc.vector.tensor_copy(out=dest[:, 512:1024], in_=ps1[:])
                yield "dest"

        def _pull(it, n):
            for _ in range(n):
                try:
                    next(it)
                except StopIteration:
                    return

        oTs = []
        qkTs = {0: (
            qkT_p.tile([P, T], MM_DT, tag="qT", name="qT0"),
            qkT_p.tile([P, T], MM_DT, tag="kT", name="kT0"),
        )}
        _pull(_proj(0, *qkTs[0], wide=True), 99)

        wo_all = wo_p.tile([P, NPAIR, T], MM_DT, tag="wo", name="wot")
        wo_r = wo_d.ap().rearrange("(n p) t -> p n t", p=P)
        for q in range(4):
            nc.sync.dma_start(
                out=wo_all[:, 2 * q:2 * q + 2, :], in_=wo_r[:, 2 * q:2 * q + 2, :]
            )
        wots = [wo_all[:, g, :] for g in range(NPAIR)]
        op_accs = None
        rcps = {}
        parts = {}

        def _norm_chunk(oT, rcpg, c, bc):
            nc.tensor.matmul(
                bc[:], sel2[:], rcpg[c][0:2, 0:512],
                start=True, stop=True,
            )
            nc.vector.tensor_tensor(
                oT[:, c * 512:(c + 1) * 512],
                oT[:, c * 512:(c + 1) * 512],
                bc[:],
                MULT,
            )

        def _p7_weave():
            # out-proj t-tile 0, pairs 0..5
            for gg in range(6):
                for c in range(2):
                    nc.tensor.matmul(
                        op_accs[c][:],
                        oTs[gg][:, 0:P],
                        wots[gg][:, c * 512:(c + 1) * 512],
                        start=(gg == 0), stop=False,
                    )
                yield "op"

            def _partial(ptt, c):
                pb = psAV.tile([P, 512], F32, tag="av", name=f"pb{ptt}{c}")
                for gg in range(6):
                    nc.tensor.matmul(
                        pb[:],
                        oTs[gg][:, ptt * P:(ptt + 1) * P],
                        wots[gg][:, c * 512:(c + 1) * 512],
                        start=(gg == 0), stop=(gg == 5),
                    )
                sb = part_p.tile([P, 512], F32, tag="part", name=f"pt{ptt}{c}")
                parts[ptt, c] = sb
                nc.vector.tensor_scalar(
                    sb[:], pb[:], mcol[:, ptt:ptt + 1], None, MULT,
                )

            # partial out-proj (pairs 0..5, masked) for t-tiles 1..3, with
            # pair-6 normalization + its t-tile-0 matmuls slotted between
            _partial(PART_TTS[0], 0)
            yield "p"
            _partial(PART_TTS[0], 1)
            yield "p"
            for c in range(2):
                bc = psAV.tile([P, 512], F32, tag="av", name=f"nbc6_{c}")
                _norm_chunk(oTs[6], rcps[6], c, bc)
            yield "n6"
            _partial(PART_TTS[1], 0)
            yield "p"
            _partial(PART_TTS[1], 1)
            yield "p"
            for c in range(2):
                nc.tensor.matmul(
                    op_accs[c][:],
                    oTs[6][:, 0:P],
                    wots[6][:, c * 512:(c + 1) * 512],
                    start=False, stop=False,
                )
            yield "op6"
            _partial(PART_TTS[2], 0)
            yield "p"
            _partial(PART_TTS[2], 1)
            yield "p"

        for g in range(NPAIR):
            qT, kT = qkTs[g]
            last = g == NPAIR - 1
            if not last:
                qkTs[g + 1] = (
                    qkT_p.tile([P, T], MM_DT, tag="qT", name=f"qT{g + 1}"),
                    qkT_p.tile([P, T], MM_DT, tag="kT", name=f"kT{g + 1}"),
                )
                nxt = _proj(g + 1, *qkTs[g + 1])
            else:
                op_accs = {
                    c: psA.tile([P, 512], F32, tag="ps", name=f"ops0_{c}")
                    for c in range(2)
                }
                nxt = _p7_weave()

            oT = oT_p.tile([P, T], MM_DT, tag=f"oT{g}", name=f"oT{g}")
            oTs.append(oT)
            # separate tiles per ci-chunk: chunk 0's normalize matmul must
            # not pick up a dependency on chunk 1's reciprocal chain
            deng = {ci: den_p.tile([1, 2, 512], F32, tag=f"deng{ci}",
                                   name=f"deng{g}_{ci}") for ci in (0, 1)}
            den2 = {ci: den_p.tile([2, 512], F32, tag=f"den2_{ci}",
                                   name=f"den2_{g}_{ci}") for ci in (0, 1)}
            rf32 = {ci: den_p.tile([2, 512], F32, tag=f"rf32_{ci}",
                                   name=f"rf32_{g}_{ci}") for ci in (0, 1)}
            rcpg = {ci: den_p.tile([2, 512], MM_DT, tag=f"rcp{ci}",
                                   name=f"rcp{g}_{ci}") for ci in (0, 1)}
            rcps[g] = rcpg

            # dense score block, exp-paced: weave fills the ACT gaps
            ptts = {}
            for J in range(NT):
                ptts[J] = pt_p.tile([P, 2, T], MM_DT, tag="pt", name=f"pt{J}")
                for (lo, w) in _qk_chunks(J):
                    ps = psS.tile([P, 1024], F32, tag="s", name="sps")
                    for hh in (0, 1):
                        hs = slice(hh * DH, (hh + 1) * DH)
                        nc.tensor.matmul(
                            ps[:, hh * 512:hh * 512 + w],
                            kT[hs, J * P:(J + 1) * P],
                            qT[hs, lo:lo + w],
                            start=True, stop=True,
                        )
                    nc.scalar.activation(
                        out=ptts[J][:, :, lo:lo + w],
                        in_=ps[:].rearrange("p (h i) -> p h i", h=2)[:, :, :w],
                        func=EXP, scale=SCALE,
                    )
                    _pull(nxt, 1)
                nc.vector.tensor_tensor(
                    ptts[J][:, :, J * P:(J + 1) * P],
                    ptts[J][:, :, J * P:(J + 1) * P],
                    tri[:].rearrange("p (o j) -> p o j", o=1)
                          .to_broadcast([P, 2, P]),
                    MULT,
                )
                if J < 6:
                    _pull(nxt, 1)

            # dense A@V, ci-outer like the baseline (2 rotating banks, the
            # bank-reuse WAR is one whole block behind its evacuation). oT
            # evacuation on ACT (it idles once the exps drain), denominator
            # rows + reciprocal per ci-chunk on DVE.
            def _recip_chunk(ci):
                nc.sync.dma_start(out=den2[ci][:], in_=deng[ci][0:1, :, :])
                nc.vector.reciprocal_approx_fast(
                    out=rf32[ci][:], in_=den2[ci][:]
                )
                with nc.allow_low_precision(reason="fp16 recip for matmul"):
                    nc.vector.tensor_copy(out=rcpg[ci][:], in_=rf32[ci][:])

            def _av_block(hh, ci):
                h = 2 * g + hh
                clo = ci * 512
                acc = psAV.tile([P, 512], F32, tag="av", name=f"av{hh}{ci}")
                Js = range(4) if ci == 0 else range(NT)
                for J in Js:
                    lo = max(clo, J * P)
                    w = clo + 512 - lo
                    nc.tensor.matmul(
                        acc[0:DH + 1, lo - clo:lo - clo + w],
                        vaug[J][:, h, :],
                        ptts[J][:, hh, lo:lo + w],
                        start=(J == 0), stop=(J == Js[-1]),
                    )
                return acc

            def _av_evac(acc, hh, ci, on_act):
                # the bank-reuse WAR gates later PE work through this
                # evacuation, so it goes on whichever engine is idle at this
                # point of the pair: DVE mid-pair (ACT still drains exps),
                # ACT at pair end (exps done, DVE has the recip chain)
                hs = slice(hh * DH, (hh + 1) * DH)
                clo = ci * 512
                if on_act:
                    nc.scalar.copy(
                        out=oT[hs, clo:clo + 512], in_=acc[0:DH, 0:512],
                    )
                else:
                    nc.vector.tensor_copy(
                        out=oT[hs, clo:clo + 512], in_=acc[0:DH, 0:512],
                    )
                nc.vector.tensor_copy(
                    out=deng[ci][0:1, hh, 0:512],
                    in_=acc[DH:DH + 1, 0:512],
                )

            acc = _av_block(0, 0)
            _av_evac(acc, 0, 0, on_act=not last)
            _pull(nxt, 1)
            acc = _av_block(1, 0)
            _av_evac(acc, 1, 0, on_act=not last)
            _recip_chunk(0)
            _pull(nxt, 1)
            acc = _av_block(0, 1)
            if last:
                # normalize pair 7's first oT chunk NOW: its inputs (ci=0
                # evacuations + reciprocal) are long done, and emitting it
                # before av01's evacuation keeps its counting-sem thresholds
                # low so the in-order PE queue never stalls here
                bc = psAV.tile([P, 512], F32, tag="av", name="nbc7_0")
                _norm_chunk(oT, rcps[7], 0, bc)
            _av_evac(acc, 0, 1, on_act=not last)
            _pull(nxt, 1)
            acc = _av_block(1, 1)
            _av_evac(acc, 1, 1, on_act=not last)
            _recip_chunk(1)
            _pull(nxt, 99)

            # normalize the PREVIOUS pair (reciprocal long ready; pair 7
            # normalizes pair 6 inline in its weave instead)
            if 1 <= g < NPAIR - 1:
                for c in range(2):
                    bc = psA.tile([P, 512], F32, tag="ps", name=f"nbc{g}_{c}")
                    _norm_chunk(oTs[g - 1], rcps[g - 1], c, bc)

        # ---- Phase 3: output projection.
        def _op_finish(tt, accs):
            for c in range(2):
                osb = osb_p.tile([P, 512], F32, tag="osb", name="osb")
                if (tt, c) in parts:
                    nc.vector.scalar_tensor_tensor(
                        out=osb[:], in0=accs[c][:], scalar=mcol[:, tt:tt + 1],
                        in1=parts[tt, c][:], op0=MULT, op1=ADD,
                    )
                else:
                    nc.vector.tensor_scalar(
                        osb[:], accs[c][:], mcol[:, tt:tt + 1], None, MULT,
                    )
                nc.sync.dma_start(
                    out=out_d.ap()[tt * P:(tt + 1) * P,
                                   c * 512:(c + 1) * 512],
                    in_=osb[:],
                )

        def _op_alloc():
            return {
                c: (psA if c == 0 else psAV).tile(
                    [P, 512], F32, tag=("ps" if c == 0 else "av"),
                    name=f"ops{c}",
                )
                for c in range(2)
            }

        # t-tile 0 finishes immediately (oT7 cols < 128 normalized in-pair)
        for c in range(2):
            nc.tensor.matmul(
                op_accs[c][:], oTs[7][:, 0:P],
                wots[7][:, c * 512:(c + 1) * 512],
                start=False, stop=True,
            )
        _op_finish(0, op_accs)

        # partial t-tiles: resume with pairs 6..7 only
        for tt in PART_TTS:
            accs = _op_alloc()
            for gg in (6, 7):
                for c in range(2):
                    nc.tensor.matmul(
                        accs[c][:],
                        oTs[gg][:, tt * P:(tt + 1) * P],
                        wots[gg][:, c * 512:(c + 1) * 512],
                        start=(gg == 6), stop=(gg == 7),
                    )
            _op_finish(tt, accs)

        bc = psAV.tile([P, 512], F32, tag="av", name="nbc7_1")
        _norm_chunk(oTs[7], rcps[7], 1, bc)

        for tt in range(PART_TTS[-1] + 1, NT):
            accs = _op_alloc()
            for gg in range(NPAIR):
                for c in range(2):
                    nc.tensor.matmul(
                        accs[c][:],
                        oTs[gg][:, tt * P:(tt + 1) * P],
                        wots[gg][:, c * 512:(c + 1) * 512],
                        start=(gg == 0), stop=(gg == NPAIR - 1),
                    )
            _op_finish(tt, accs)


def build_nc():
    nc = bacc.Bacc("TRN2", target_bir_lowering=False, debug=False,
                   num_devices=8)
    xT_d = nc.dram_tensor("xT", [D, T], MM_DT, kind="ExternalInput")
    wqk_d = nc.dram_tensor("wqk", [H, ND, P, P], MM_DT, kind="ExternalInput")
    wv_d = nc.dram_tensor("wv", [D, D], MM_DT, kind="ExternalInput")
    wo_d = nc.dram_tensor("wo", [D, D], MM_DT, kind="ExternalInput")
    mcol_d = nc.dram_tensor("mcol", [P, NT], F32, kind="ExternalInput")
    tri_d = nc.dram_tensor("tri", [P, P], MM_DT, kind="ExternalInput")
    sel2_d = nc.dram_tensor("sel2", [2, P], MM_DT, kind="ExternalInput")
    out_d = nc.dram_tensor("out", [T, D], F32, kind="ExternalOutput")
    with tile.TileContext(nc) as tc:
        _emit(nc, tc, xT_d, wqk_d, wv_d, wo_d, mcol_d, tri_d, sel2_d, out_d)
    nc.compile()
    return nc


def _prep_shared(w_qkv, w_out):
    wqkT = np.ascontiguousarray(w_qkv[:2 * D].T)             # [d, e]
    wqk_tiles = np.ascontiguousarray(
        wqkT.reshape(ND, P, H, P).transpose(2, 0, 1, 3)
    ).astype(NP_MM)                                          # [16, 8, 128, 128]
    wv = np.ascontiguousarray(w_qkv[2 * D:].T).astype(NP_MM)  # [d, ev]
    wo = np.ascontiguousarray(w_out.T).astype(NP_MM)          # [d', e]
    tri = np.triu(np.ones((P, P), dtype=np.float32)).astype(NP_MM)
    sel2 = np.zeros((2, P), dtype=np.float32)
    sel2[0, 0:DH] = 1.0
    sel2[1, DH:P] = 1.0
    sel2 = sel2.astype(NP_MM)
    return wqk_tiles, wv, wo, tri, sel2


def kernel(x, m, w_qkv, w_out, b_out, l=None, **_unused):
    global LAST_RESULTS
    x = np.asarray(x, dtype=np.float32)
    m = np.asarray(m, dtype=np.float32)
    w_qkv = np.asarray(w_qkv, dtype=np.float32)
    w_out = np.asarray(w_out, dtype=np.float32)
    b_out = np.asarray(b_out, dtype=np.float32)

    if "nc" not in _CACHE:
        _CACHE["nc"] = build_nc()
    nc = _CACHE["nc"]

    wqk_tiles, wv, wo, tri, sel2 = _prep_shared(w_qkv, w_out)
    in_maps = []
    for b in range(8):
        in_maps.append({
            "xT": np.ascontiguousarray(x[b].T).astype(NP_MM),
            "wqk": wqk_tiles,
            "wv": wv,
            "wo": wo,
            "mcol": np.ascontiguousarray(m[b].reshape(NT, P).T),
            "tri": tri,
            "sel2": sel2,
        })

    trace = bool(int(os.environ.get("TRN_TRACE", "0")))
    res = run_bass_kernel_spmd(
        nc, in_maps, core_ids=list(range(8)), trace=trace,
    )
    LAST_RESULTS = res
    out = np.stack([res.results[b]["out"] for b in range(8)], axis=0)
    # bias is a broadcast add; do it on the host (masked like the reference)
    out = out + b_out[None, None, :] * m[:, :, None]
    return out.astype(np.float32)


# revision 28
# speedup vs baseline: 1.2077x; 1.0112x over previous
"""Trainium2 Bass kernel for a causal self-attention transformer block.

Reference computation (per batch b):
    qkv = x @ w_qkv.T ; split into q, k, v heads (16 heads, dim 64)
    s   = (q @ k.T) * dh**-0.5, causal + padding mask
    a   = softmax(s, axis=j)
    o   = (a @ v) @ w_out.T + b_out ; out = o * m[:, None]

Sharding: pure data parallel — batch (8) across the 8 NeuronCores, weights
replicated. No collectives.

Per-core device program (v3):
  - host-pre-transposed inputs: xT [d, t], wqk tiled [16, 8, 128, 128]
    (lhsT tiles), wv/wo [d, e]; all matmul operands fp16, accumulation fp32
    in PSUM.
  - qT/kT in [e, t] layout (2 heads per 128-partition tile); v in [t, e]
    layout with a padding-mask column so A@V also emits the softmax
    denominator row.
  - scores transposed: S_T[j, i] = K^T.T @ Q^T; the two heads' K=64 matmuls
    stream concurrently in row-groups 0-1/2-3 into one 2-bank PSUM tile
    (double-buffered), exp is a single ACTIVATE per chunk over both heads,
    causal tri-mask one DVE multiply per j-tile. The score loop is filler-
    woven with the next pair's q/k projection because exp paces it.
  - A@V dense blocks, hh-outer J-major: 2 live accumulators, stationary
    shared across the two i-chunks per (head, J) so half the LDWEIGHTS
    disappear; denominator rows staged on partition 0, tiny DMA to
    partitions 0/1, reciprocal_approx_fast, fp16 cast; normalization
    broadcast via a K=2 fp16 sel2-matmul one pair late.
  - pair 7 has no next projection to weave, so it weaves the first out-proj
    t-tile AND partial out-proj accumulations (pairs 0-5) for t-tiles 1-3,
    evacuated masked to SBUF; phase 3 resumes them with a fused
    (psum*mask)+partial DVE op. Pair 6/7 oT normalization is chunked so
    out-proj starts immediately — the PE never idles long enough for the
    HAM clock-gate to re-throttle.
  - b_out is added on the host (removes 16 fp32r K=1 bias matmuls).
"""

import os
import numpy as np
from contextlib import ExitStack

import ml_dtypes
from concourse import bacc
import concourse.mybir as mybir
import concourse.tile as tile
from concourse.bass_utils import run_bass_kernel_spmd

D = 1024          # model dim
T = 1024          # sequence length
H = 16            # heads
DH = 64           # head dim
P = 128           # partitions
ND = D // P       # d-tiles
NT = T // P       # t-tiles
NPAIR = H // 2    # head pairs
SCALE = DH ** -0.5
F32 = mybir.dt.float32
MULT = mybir.AluOpType.mult
ADD = mybir.AluOpType.add
EXP = mybir.ActivationFunctionType.Exp

_MM_MODE = os.environ.get("TRN_MM_DT", "fp16")
MM_DT = {"fp16": mybir.dt.float16, "bf16": mybir.dt.bfloat16}[_MM_MODE]
NP_MM = {"fp16": np.float16, "bf16": ml_dtypes.bfloat16}[_MM_MODE]

_CACHE = {}
LAST_RESULTS = None
PART_TTS = (1, 2, 3)   # t-tiles whose pair-0..5 out-proj is woven into pair 7


def _qk_chunks(J):
    """i-column chunks (lo, width) of computed scores for j-tile J."""
    out = []
    for lo in (J * P, J * P + 512):
        w = min(512, T - lo)
        if w > 0:
            out.append((lo, w))
    return out


def _av_ranges(J):
    """(ci, lo, width) A@V output ranges for j-tile J."""
    out = []
    if J < 4:
        out.append((0, J * P, 512 - J * P))
    lo = max(512, J * P)
    out.append((1, lo, T - lo))
    return out


def _emit(nc, tc, xT_d, wqk_d, wv_d, wo_d, mcol_d, tri_d, sel2_d, out_d):
    ctx = ExitStack()
    with ctx:
        const = ctx.enter_context(tc.tile_pool(name="const", bufs=1))
        xt_p = ctx.enter_context(tc.tile_pool(name="xt", bufs=1))
        vaug_p = ctx.enter_context(tc.tile_pool(name="vaug", bufs=1))
        qkT_p = ctx.enter_context(tc.tile_pool(name="qkT", bufs=2))
        wqk_p = ctx.enter_context(tc.tile_pool(name="wqk", bufs=4))
        pt_p = ctx.enter_context(tc.tile_pool(name="pt", bufs=10))
        oT_p = ctx.enter_context(tc.tile_pool(name="oT", bufs=1))
        wv_p = ctx.enter_context(tc.tile_pool(name="wv", bufs=1))
        wo_p = ctx.enter_context(tc.tile_pool(name="wo", bufs=1))
        osb_p = ctx.enter_context(tc.tile_pool(name="osb", bufs=6))
        part_p = ctx.enter_context(tc.tile_pool(name="part", bufs=6))
        den_p = ctx.enter_context(tc.tile_pool(name="den", bufs=2))
        # PSUM: 8 banks = psA(2) + psS(2x2) + psAV(2)
        psA = ctx.enter_context(tc.tile_pool(name="psA", bufs=2, space="PSUM"))
        psS = ctx.enter_context(tc.tile_pool(name="psS", bufs=2, space="PSUM"))
        psAV = ctx.enter_context(tc.tile_pool(name="psAV", bufs=2, space="PSUM"))

        # resident xT and wv tiles, one tile per d-tile (separate tiles give
        # per-DMA dependency granularity so the first V-proj matmul only
        # waits for its own d-slice), DMA'd interleaved in consumption order.
        xT_r = xT_d.ap().rearrange("(n p) t -> p n t", p=P)
        wv_r = wv_d.ap().rearrange("(n p) t -> p n t", p=P)
        xt_tiles = [xt_p.tile([P, T], MM_DT, tag=f"xt{d}", name=f"xt{d}")
                    for d in range(ND)]
        wv_tiles = [wv_p.tile([P, T], MM_DT, tag=f"wv{d}", name=f"wv{d}")
                    for d in range(ND)]
        for q in range(ND):
            nc.sync.dma_start(out=xt_tiles[q][:], in_=xT_r[:, q, :])
            nc.sync.dma_start(out=wv_tiles[q][:], in_=wv_r[:, q, :])
        xts = [xt_tiles[d][:] for d in range(ND)]
        wvts = [wv_tiles[d][:] for d in range(ND)]

        mcol = const.tile([P, NT], F32, tag="mcol", name="mcol")
        nc.sync.dma_start(out=mcol[:], in_=mcol_d.ap())
        tri = const.tile([P, P], MM_DT, tag="tri", name="tri")
        nc.sync.dma_start(out=tri[:], in_=tri_d.ap())
        sel2 = const.tile([2, P], MM_DT, tag="sel2", name="sel2")
        nc.sync.dma_start(out=sel2[:], in_=sel2_d.ap())

        vaug = [
            vaug_p.tile([P, H, DH + 1], MM_DT, tag=f"va{t}", name=f"va{t}")
            for t in range(NT)
        ]

        # ---- Phase 1: V projection (natural layout).
        for g2 in range(0, NT, 2):
            accs = {}
            for i in range(2):
                for c in range(2):
                    pool = psA if i == 0 else psAV
                    accs[i, c] = pool.tile(
                        [P, 512], F32, tag=("ps" if i == 0 else "av"),
                        name=f"vps{i}{c}",
                    )
            for d in range(ND):
                for i in range(2):
                    tt = g2 + i
                    for c in range(2):
                        nc.tensor.matmul(
                            accs[i, c][:],
                            xts[d][:, tt * P:(tt + 1) * P],
                            wvts[d][:, c * 512:(c + 1) * 512],
                            start=(d == 0),
                            stop=(d == ND - 1),
                        )
            for i in range(2):
                tt = g2 + i
                for c in range(2):
                    ps3 = accs[i, c][:].rearrange("p (h e) -> p h e", e=DH)
                    nc.vector.tensor_scalar(
                        vaug[tt][:, c * 8:(c + 1) * 8, 0:DH],
                        ps3,
                        mcol[:, tt:tt + 1],
                        None,
                        MULT,
                    )
        for tt in range(NT):
            nc.vector.tensor_copy(
                out=vaug[tt][:, :, DH],
                in_=mcol[:, tt:tt + 1].to_broadcast([P, H]),
            )

        # ---- Phase 2: per head-pair: q/k projection then attention.
        def _proj(g, qT, kT, wide=False):
            """Generator emitting pair g's q/k projection in small steps.
            wide=True interleaves the q and k chains on 4 accumulators
            (only legal when psAV is otherwise idle)."""
            if wide:
                wts, pss = [], []
                for et in (g, NPAIR + g):
                    wt = wqk_p.tile([P, ND, P], MM_DT, tag="wqk", name="wqkt")
                    nc.sync.dma_start(
                        out=wt[:], in_=wqk_d.ap()[et].rearrange("n p e -> p n e")
                    )
                    wts.append(wt)
                    pool = psA if et == g else psAV
                    tg = "ps" if et == g else "av"
                    pss.append([
                        pool.tile([P, 512], F32, tag=tg, name="qkps")
                        for _ in range(2)
                    ])
                for d in range(ND):
                    for k in range(2):
                        for half in range(2):
                            nc.tensor.matmul(
                                pss[k][half][:],
                                wts[k][:, d, :],
                                xts[d][:, half * 512:(half + 1) * 512],
                                start=(d == 0), stop=(d == ND - 1),
                            )
                    yield "d"
                for k, dest in ((0, qT), (1, kT)):
                    for half in range(2):
                        nc.vector.tensor_copy(
                            out=dest[:, half * 512:(half + 1) * 512],
                            in_=pss[k][half][:],
                        )
                    yield "dest"
                return
            for dest, et in ((qT, g), (kT, NPAIR + g)):
                wt = wqk_p.tile([P, ND, P], MM_DT, tag="wqk", name="wqkt")
                nc.sync.dma_start(
                    out=wt[:], in_=wqk_d.ap()[et].rearrange("n p e -> p n e")
                )
                ps0 = psA.tile([P, 512], F32, tag="ps", name="qkps0")
                ps1 = psA.tile([P, 512], F32, tag="ps", name="qkps1")
                for d in range(ND):
                    nc.tensor.matmul(
                        ps0[:], wt[:, d, :], xts[d][:, 0:512],
                        start=(d == 0), stop=(d == ND - 1),
                    )
                    nc.tensor.matmul(
                        ps1[:], wt[:, d, :], xts[d][:, 512:1024],
                        start=(d == 0), stop=(d == ND - 1),
                    )
                    yield "d"
                nc.vector.tensor_copy(out=dest[:, 0:512], in_=ps0[:])
                nc.vector.tensor_copy(out=dest[:, 512:1024], in_=ps1[:])
                yield "dest"

        def _pull(it, n):
            for _ in range(n):
                try:
                    next(it)
                except StopIteration:
                    return

        oTs = []
        qkTs = {0: (
            qkT_p.tile([P, T], MM_DT, tag="qT", name="qT0"),
            qkT_p.tile([P, T], MM_DT, tag="kT", name="kT0"),
        )}
        _pull(_proj(0, *qkTs[0], wide=True), 99)

        wo_all = wo_p.tile([P, NPAIR, T], MM_DT, tag="wo", name="wot")
        wo_r = wo_d.ap().rearrange("(n p) t -> p n t", p=P)
        for q in range(4):
            nc.sync.dma_start(
                out=wo_all[:, 2 * q:2 * q + 2, :], in_=wo_r[:, 2 * q:2 * q + 2, :]
            )
        wots = [wo_all[:, g, :] for g in range(NPAIR)]
        op_accs = None
        rcps = {}
        parts = {}

        def _norm_chunk(oT, rcpg, c, bc):
            nc.tensor.matmul(
                bc[:], sel2[:], rcpg[c][0:2, 0:512],
                start=True, stop=True,
            )
            nc.vector.tensor_tensor(
                oT[:, c * 512:(c + 1) * 512],
                oT[:, c * 512:(c + 1) * 512],
                bc[:],
                MULT,
            )

        def _p7_weave():
            # out-proj t-tile 0, pairs 0..5
            for gg in range(6):
                for c in range(2):
                    nc.tensor.matmul(
                        op_accs[c][:],
                        oTs[gg][:, 0:P],
                        wots[gg][:, c * 512:(c + 1) * 512],
                        start=(gg == 0), stop=False,
                    )
                yield "op"

            def _partial(ptt, c):
                pb = psAV.tile([P, 512], F32, tag="av", name=f"pb{ptt}{c}")
                for gg in range(6):
                    nc.tensor.matmul(
                        pb[:],
                        oTs[gg][:, ptt * P:(ptt + 1) * P],
                        wots[gg][:, c * 512:(c + 1) * 512],
                        start=(gg == 0), stop=(gg == 5),
                    )
                sb = part_p.tile([P, 512], F32, tag="part", name=f"pt{ptt}{c}")
                parts[ptt, c] = sb
                nc.vector.tensor_scalar(
                    sb[:], pb[:], mcol[:, ptt:ptt + 1], None, MULT,
                )

            # partial out-proj (pairs 0..5, masked) for t-tiles 1..3, with
            # pair-6 normalization + its t-tile-0 matmuls slotted between
            _partial(PART_TTS[0], 0)
            yield "p"
            _partial(PART_TTS[0], 1)
            yield "p"
            for c in range(2):
                bc = psAV.tile([P, 512], F32, tag="av", name=f"nbc6_{c}")
                _norm_chunk(oTs[6], rcps[6], c, bc)
            yield "n6"
            _partial(PART_TTS[1], 0)
            yield "p"
            _partial(PART_TTS[1], 1)
            yield "p"
            for c in range(2):
                nc.tensor.matmul(
                    op_accs[c][:],
                    oTs[6][:, 0:P],
                    wots[6][:, c * 512:(c + 1) * 512],
                    start=False, stop=False,
                )
            yield "op6"
            _partial(PART_TTS[2], 0)
            yield "p"
            _partial(PART_TTS[2], 1)
            yield "p"

        for g in range(NPAIR):
            qT, kT = qkTs[g]
            last = g == NPAIR - 1
            if not last:
                qkTs[g + 1] = (
                    qkT_p.tile([P, T], MM_DT, tag="qT", name=f"qT{g + 1}"),
                    qkT_p.tile([P, T], MM_DT, tag="kT", name=f"kT{g + 1}"),
                )
                nxt = _proj(g + 1, *qkTs[g + 1])
            else:
                op_accs = {
                    c: psA.tile([P, 512], F32, tag="ps", name=f"ops0_{c}")
                    for c in range(2)
                }
                nxt = _p7_weave()

            oT = oT_p.tile([P, T], MM_DT, tag=f"oT{g}", name=f"oT{g}")
            oTs.append(oT)
            # separate tiles per ci-chunk: chunk 0's normalize matmul must
            # not pick up a dependency on chunk 1's reciprocal chain
            deng = {ci: den_p.tile([1, 2, 512], F32, tag=f"deng{ci}",
                                   name=f"deng{g}_{ci}") for ci in (0, 1)}
            den2 = {ci: den_p.tile([2, 512], F32, tag=f"den2_{ci}",
                                   name=f"den2_{g}_{ci}") for ci in (0, 1)}
            rf32 = {ci: den_p.tile([2, 512], F32, tag=f"rf32_{ci}",
                                   name=f"rf32_{g}_{ci}") for ci in (0, 1)}
            rcpg = {ci: den_p.tile([2, 512], MM_DT, tag=f"rcp{ci}",
                                   name=f"rcp{g}_{ci}") for ci in (0, 1)}
            rcps[g] = rcpg

            # dense score block, exp-paced: weave fills the ACT gaps
            ptts = {}
            for J in range(NT):
                ptts[J] = pt_p.tile([P, 2, T], MM_DT, tag="pt", name=f"pt{J}")
                for (lo, w) in _qk_chunks(J):
                    ps = psS.tile([P, 1024], F32, tag="s", name="sps")
                    for hh in (0, 1):
                        hs = slice(hh * DH, (hh + 1) * DH)
                        nc.tensor.matmul(
                            ps[:, hh * 512:hh * 512 + w],
                            kT[hs, J * P:(J + 1) * P],
                            qT[hs, lo:lo + w],
                            start=True, stop=True,
                        )
                    nc.scalar.activation(
                        out=ptts[J][:, :, lo:lo + w],
                        in_=ps[:].rearrange("p (h i) -> p h i", h=2)[:, :, :w],
                        func=EXP, scale=SCALE,
                    )
                    _pull(nxt, 1)
                nc.vector.tensor_tensor(
                    ptts[J][:, :, J * P:(J + 1) * P],
                    ptts[J][:, :, J * P:(J + 1) * P],
                    tri[:].rearrange("p (o j) -> p o j", o=1)
                          .to_broadcast([P, 2, P]),
                    MULT,
                )
                if J < 6:
                    _pull(nxt, 1)

            # dense A@V, ci-outer like the baseline (2 rotating banks, the
            # bank-reuse WAR is one whole block behind its evacuation). oT
            # evacuation on ACT (it idles once the exps drain), denominator
            # rows + reciprocal per ci-chunk on DVE.
            def _recip_chunk(ci):
                nc.sync.dma_start(out=den2[ci][:], in_=deng[ci][0:1, :, :])
                nc.vector.reciprocal_approx_fast(
                    out=rf32[ci][:], in_=den2[ci][:]
                )
                with nc.allow_low_precision(reason="fp16 recip for matmul"):
                    nc.vector.tensor_copy(out=rcpg[ci][:], in_=rf32[ci][:])

            def _av_block(hh, ci):
                h = 2 * g + hh
                clo = ci * 512
                acc = psAV.tile([P, 512], F32, tag="av", name=f"av{hh}{ci}")
                Js = range(4) if ci == 0 else range(NT)
                for J in Js:
                    lo = max(clo, J * P)
                    w = clo + 512 - lo
                    nc.tensor.matmul(
                        acc[0:DH + 1, lo - clo:lo - clo + w],
                        vaug[J][:, h, :],
                        ptts[J][:, hh, lo:lo + w],
                        start=(J == 0), stop=(J == Js[-1]),
                    )
                return acc

            def _av_evac(acc, hh, ci, on_act):
                # the bank-reuse WAR gates later PE work through this
                # evacuation, so it goes on whichever engine is idle at this
                # point of the pair: DVE mid-pair (ACT still drains exps),
                # ACT at pair end (exps done, DVE has the recip chain)
                hs = slice(hh * DH, (hh + 1) * DH)
                clo = ci * 512
                if on_act:
                    nc.scalar.copy(
                        out=oT[hs, clo:clo + 512], in_=acc[0:DH, 0:512],
                    )
                else:
                    nc.vector.tensor_copy(
                        out=oT[hs, clo:clo + 512], in_=acc[0:DH, 0:512],
                    )
                nc.vector.tensor_copy(
                    out=deng[ci][0:1, hh, 0:512],
                    in_=acc[DH:DH + 1, 0:512],
                )

            acc = _av_block(0, 0)
            _av_evac(acc, 0, 0, on_act=not last)
            _pull(nxt, 1)
            acc = _av_block(1, 0)
            _av_evac(acc, 1, 0, on_act=not last)
            _recip_chunk(0)
            _pull(nxt, 1)
            acc = _av_block(0, 1)
            _av_evac(acc, 0, 1, on_act=not last)
            _pull(nxt, 1)
            if last:
                # normalize pair 7's first oT chunk NOW: its inputs (ci=0
                # evacuations + reciprocal) are long done, and emitting it
                # before the ci=1 reciprocal chain keeps its counting-sem
                # threshold low so phase 3 starts with minimal stall
                bc = psAV.tile([P, 512], F32, tag="av", name="nbc7_0")
                _norm_chunk(oT, rcps[7], 0, bc)
            acc = _av_block(1, 1)
            _av_evac(acc, 1, 1, on_act=not last)
            _recip_chunk(1)
            _pull(nxt, 99)

            # normalize the PREVIOUS pair (reciprocal long ready; pair 7
            # normalizes pair 6 inline in its weave instead)
            if 1 <= g < NPAIR - 1:
                for c in range(2):
                    bc = psA.tile([P, 512], F32, tag="ps", name=f"nbc{g}_{c}")
                    _norm_chunk(oTs[g - 1], rcps[g - 1], c, bc)

        # ---- Phase 3: output projection.
        def _op_finish(tt, accs):
            for c in range(2):
                osb = osb_p.tile([P, 512], F32, tag="osb", name="osb")
                if (tt, c) in parts:
                    nc.vector.scalar_tensor_tensor(
                        out=osb[:], in0=accs[c][:], scalar=mcol[:, tt:tt + 1],
                        in1=parts[tt, c][:], op0=MULT, op1=ADD,
                    )
                else:
                    nc.vector.tensor_scalar(
                        osb[:], accs[c][:], mcol[:, tt:tt + 1], None, MULT,
                    )
                nc.sync.dma_start(
                    out=out_d.ap()[tt * P:(tt + 1) * P,
                                   c * 512:(c + 1) * 512],
                    in_=osb[:],
                )

        def _op_alloc():
            return {
                c: (psA if c == 0 else psAV).tile(
                    [P, 512], F32, tag=("ps" if c == 0 else "av"),
                    name=f"ops{c}",
                )
                for c in range(2)
            }

        # t-tile 0 finishes immediately (oT7 cols < 128 normalized in-pair)
        for c in range(2):
            nc.tensor.matmul(
                op_accs[c][:], oTs[7][:, 0:P],
                wots[7][:, c * 512:(c + 1) * 512],
                start=False, stop=True,
            )
        _op_finish(0, op_accs)

        # partial t-tiles: resume with pairs 6..7 only
        for tt in PART_TTS:
            accs = _op_alloc()
            for gg in (6, 7):
                for c in range(2):
                    nc.tensor.matmul(
                        accs[c][:],
                        oTs[gg][:, tt * P:(tt + 1) * P],
                        wots[gg][:, c * 512:(c + 1) * 512],
                        start=(gg == 6), stop=(gg == 7),
                    )
            _op_finish(tt, accs)

        bc = psAV.tile([P, 512], F32, tag="av", name="nbc7_1")
        _norm_chunk(oTs[7], rcps[7], 1, bc)

        for tt in range(PART_TTS[-1] + 1, NT):
            accs = _op_alloc()
            for gg in range(NPAIR):
                for c in range(2):
                    nc.tensor.matmul(
                        accs[c][:],
                        oTs[gg][:, tt * P:(tt + 1) * P],
                        wots[gg][:, c * 512:(c + 1) * 512],
                        start=(gg == 0), stop=(gg == NPAIR - 1),
                    )
            _op_finish(tt, accs)


def build_nc():
    nc = bacc.Bacc("TRN2", target_bir_lowering=False, debug=False,
                   num_devices=8)
    xT_d = nc.dram_tensor("xT", [D, T], MM_DT, kind="ExternalInput")
    wqk_d = nc.dram_tensor("wqk", [H, ND, P, P], MM_DT, kind="ExternalInput")
    wv_d = nc.dram_tensor("wv", [D, D], MM_DT, kind="ExternalInput")
    wo_d = nc.dram_tensor("wo", [D, D], MM_DT, kind="ExternalInput")
    mcol_d = nc.dram_tensor("mcol", [P, NT], F32, kind="ExternalInput")
    tri_d = nc.dram_tensor("tri", [P, P], MM_DT, kind="ExternalInput")
    sel2_d = nc.dram_tensor("sel2", [2, P], MM_DT, kind="ExternalInput")
    out_d = nc.dram_tensor("out", [T, D], F32, kind="ExternalOutput")
    with tile.TileContext(nc) as tc:
        _emit(nc, tc, xT_d, wqk_d, wv_d, wo_d, mcol_d, tri_d, sel2_d, out_d)
    nc.compile()
    return nc


def _prep_shared(w_qkv, w_out):
    wqkT = np.ascontiguousarray(w_qkv[:2 * D].T)             # [d, e]
    wqk_tiles = np.ascontiguousarray(
        wqkT.reshape(ND, P, H, P).transpose(2, 0, 1, 3)
    ).astype(NP_MM)                                          # [16, 8, 128, 128]
    wv = np.ascontiguousarray(w_qkv[2 * D:].T).astype(NP_MM)  # [d, ev]
    wo = np.ascontiguousarray(w_out.T).astype(NP_MM)          # [d', e]
    tri = np.triu(np.ones((P, P), dtype=np.float32)).astype(NP_MM)
    sel2 = np.zeros((2, P), dtype=np.float32)
    sel2[0, 0:DH] = 1.0
    sel2[1, DH:P] = 1.0
    sel2 = sel2.astype(NP_MM)
    return wqk_tiles, wv, wo, tri, sel2


def kernel(x, m, w_qkv, w_out, b_out, l=None, **_unused):
    global LAST_RESULTS
    x = np.asarray(x, dtype=np.float32)
    m = np.asarray(m, dtype=np.float32)
    w_qkv = np.asarray(w_qkv, dtype=np.float32)
    w_out = np.asarray(w_out, dtype=np.float32)
    b_out = np.asarray(b_out, dtype=np.float32)

    if "nc" not in _CACHE:
        _CACHE["nc"] = build_nc()
    nc = _CACHE["nc"]

    wqk_tiles, wv, wo, tri, sel2 = _prep_shared(w_qkv, w_out)
    in_maps = []
    for b in range(8):
        in_maps.append({
            "xT": np.ascontiguousarray(x[b].T).astype(NP_MM),
            "wqk": wqk_tiles,
            "wv": wv,
            "wo": wo,
            "mcol": np.ascontiguousarray(m[b].reshape(NT, P).T),
            "tri": tri,
            "sel2": sel2,
        })

    trace = bool(int(os.environ.get("TRN_TRACE", "0")))
    res = run_bass_kernel_spmd(
        nc, in_maps, core_ids=list(range(8)), trace=trace,
    )
    LAST_RESULTS = res
    out = np.stack([res.results[b]["out"] for b in range(8)], axis=0)
    # bias is a broadcast add; do it on the host (masked like the reference)
    out = out + b_out[None, None, :] * m[:, :, None]
    return out.astype(np.float32)
